# revision 24
# baseline (speedup 1.0000x reference)
"""Bass/Trainium2 kernel for nn_CopyGenerator (8-core SPMD).

Sharding: 4-way vocab (tensor parallel) x 2-way rows (data parallel).
Core c = 4*h + q owns rows [2048h, 2048h+2048) and vocab columns
[8000q, 8000q+8000).  The softmax denominator needs a cross-vocab-shard
sum: one AllReduce over 4 ranks per tapered group of row-blocks (GS),
in two independent replica groups ([[0,1,2,3],[4,5,6,7]]) that pipeline
behind compute.  The copy branch stays batch-sharded 8 ways (4
batches/core).  A tiny warmup NEFF with one AllReduce runs first, and
the main NEFF issues a junk AllReduce immediately after dispatching
its input loads: even with the warmup, the first collective inside a
NEFF pays ~12-18us of channel start that would otherwise stall the
group-0 softmax scale (and, through exp-pool backpressure, the PE).

The big matmul runs in fp8 e4m3 with perf_mode=DoubleRow (2 fp8
weights/PE cell, K=256 per pass -> ~2x bf16 FLOP rate).  hidden is
scaled x16 and W x64 before the e4m3 cast so the bulk of both
distributions sits in the normal range (min normal 2^-6); the Exp
activation un-scales via its fp32 `scale` operand (1/1024).  fp8
quantization adds ~3-4% relative noise to individual softmax probs,
which is far inside the 2e-2 budget because gen-branch probs (~2e-4)
are tiny against the copy-branch absmax (~0.1) and the denominator
noise averages out over 32000 terms.

The copy-gate logit is FOLDED into the big matmul as one extra W
column (col 8000 of the padded shard): sigmoid(x) = ep/(1+ep) with
ep = exp(x) falling out of the same Exp pass, so the per-block scale
is m = (1-gate)/S = 1/((1+ep*e^{b_copy}) * (S_allreduce - pad_corr)).
This removes 64 N=1 gate matmuls (each paying a full LDWEIGHTS) from
the PE stream.

Per 128-row block (steady state, all three engines within ~0.5us of
each other -- PE ~9.0us, ACT ~8.5us, DVE ~7.8us):
  - PE: logits into PSUM, 6 chunks of <=1536 cols ([128,1536]f32 = 3
    PSUM banks, pool of 2), 2 DoubleRow K-passes x 512-col matmuls.
  - ACT: Exp (scale=1/1024) into SBUF fp16.  Free-dim partial sums
    split between ACT accum_out (4 chunks) and DVE reduce (2 chunks):
    either engine alone would be over budget.
  - After the group all-reduce: DVE scales exp by (1-gate)/S into bf16
    staging tiles (host upcasts; probs are ~1e-4 so bf16 rounding is
    ~1e-7 absolute).
  - PAD masking: host zeroes W[PAD,:] (dead data in the reference), the
    resulting constant exp(0)=1 is subtracted from the reduced sum, and
    the host zeroes output column PAD.
  - Copy branch: fp16 matmul (one-hot src_map is exact in fp16); its
    gate ALSO needs ~1e-3 accuracy (it multiplies values that ARE the
    output absmax) so it gets its own fp16 dot product -- fp16 is
    ~5e-4 there, e4m3 would be ~2-4%.  Its PSUM tile carries both the
    600-col scatter matmul and the 1-col gate dot (disjoint psum
    regions of one [128,608] tile) so the main loop can use 6 banks.
"""

import os
import sys

for _p in ("/opt/trn_rl_repo", "/root/.axon_site/_ro/trn_rl_repo"):
    if os.path.isdir(_p) and _p not in sys.path:
        sys.path.insert(0, _p)

import numpy as np
import ml_dtypes

import concourse.bacc as bacc
import concourse.tile as tile
from concourse import mybir
from concourse.bass_utils import run_bass_kernel_spmd

# ---------------------------------------------------------------------------
# Problem dimensions (hardcoded per spec)
# ---------------------------------------------------------------------------
B, T, S, V, CV, D = 32, 128, 400, 32000, 600, 512
PAD = 1
NCORES = 8
NQ = 4                    # vocab shards
NH = 2                    # row halves
R = B * T                 # 4096 rows
VS = V // NQ              # 8000 vocab columns per core
VSP = VS + 16             # padded shard: col VS = w_copy, cols VS+1.. = 0
RH = R // NH              # 2048 rows per core
RB = 128                  # rows per block (= one batch: T == 128)
NBL = RH // RB            # 16 row blocks per core
# tapered all-reduce groups: small first group fills the pipeline before the
# exp pool saturates; tiny last groups shrink the drain tail
GS = [2, 3, 3, 3, 4, 1]   # sums to NBL
NG = len(GS)
GOFF = [sum(GS[:i]) for i in range(NG)]
GRPOF = []                # block -> (group, index-in-group)
for _g, _n in enumerate(GS):
    for _j in range(_n):
        GRPOF.append((_g, _j))
LB = B // NCORES          # 4 local batches per core (copy branch)
KC = D // 128             # 4 contraction chunks = 2 DoubleRow passes
NKK = KC // 2
# vocab chunking within a block ([128,1536]f32 = 3 PSUM banks)
NVC = 6
VCH = [1536] * 5 + [336]  # matmul/psum width (last: 320 vocab + gate + pad)
RW = [1536] * 5 + [320]   # softmax width (excludes gate + pads)
VOFF = [1536 * i for i in range(NVC)]
GCOL = 320                # gate column within chunk 5
ACT_ACC = (0, 2, 4, 5)    # chunks whose partial sum runs on ACT accum_out
# chunk -> list of (src_off, half, dst_off, width) for the two output halves
SEG = []
for _c in range(NVC):
    _s = []
    for (_h0, _h1, _hf) in ((0, 4096, 0), (4096, VS, 1)):
        _lo = max(VOFF[_c], _h0)
        _hi = min(VOFF[_c] + RW[_c], _h1)
        if _hi > _lo:
            _s.append((_lo - VOFF[_c], _hf, _lo - 4096 * _hf, _hi - _lo))
    SEG.append(_s)
# s-dim chunks for the copy branch: 400 = 128+128+128+16
SCH = [128, 128, 128, 16]
SOFF = [0, 128, 256, 384]

F32 = mybir.dt.float32
F16 = mybir.dt.float16
BF16 = mybir.dt.bfloat16
F8 = mybir.dt.float8e4
DR = mybir.MatmulPerfMode.DoubleRow

# fp8 pre-scales (host multiplies before the e4m3 cast; Exp un-scales)
SH = 16.0                 # hidden scale
SW = 64.0                 # W / w_copy scale
INV = 1.0 / (SH * SW)     # 1/1024

EXP_BUFS = 40   # in-flight exp tiles ([128,1536] f16) ~ 6.7 blocks
OUT_BUFS = 2    # [128, 4096] bf16 output staging tiles (2 per block)


def _mm_splits(n):
    """Split a free-dim span into <=512 pieces aligned to 512 (PSUM banks)."""
    out = []
    off = 0
    while off < n:
        w = min(512, n - off)
        out.append((off, w))
        off += w
    return out


def build_program(with_bias: bool, b_copy: float, pad_corr: float):
    # Bacc (not plain Bass): its finalize() runs move_matmul_waits_to_ldweights
    # + generate_event_semaphores, which split multi-sem waits down to the
    # TRN2 limit of one wait per instruction — walrus rejects the IR otherwise.
    nc = bacc.Bacc()

    ebc = float(np.exp(b_copy))

    # k-pair-packed fp8 operands: [128, 2, free] per DoubleRow pass
    h8d = [nc.dram_tensor(f"h8_{kk}", [128, 2, RH], F8, kind="ExternalInput")
           for kk in range(NKK)]
    w8d = [nc.dram_tensor(f"w8_{kk}", [128, 2, VSP], F8, kind="ExternalInput")
           for kk in range(NKK)]
    h16d = nc.dram_tensor("h16", [D, LB * RB], F16, kind="ExternalInput")
    wc16d = nc.dram_tensor("wc16", [D, 1], F16, kind="ExternalInput")
    attnT = nc.dram_tensor("attnT", [S, LB * RB], F16, kind="ExternalInput")
    smap = nc.dram_tensor("smap", [LB, S, CV], F16, kind="ExternalInput")
    if with_bias:
        ebb = nc.dram_tensor("ebb", [128, VS], F32, kind="ExternalInput")

    og = nc.dram_tensor("og", [RH, VS], BF16, kind="ExternalOutput")
    oc = nc.dram_tensor("oc", [LB * RB, CV], F32, kind="ExternalOutput")

    with tile.TileContext(nc) as tc:
        with (
            tc.tile_pool(name="const", bufs=1) as const,
            tc.tile_pool(name="pm", bufs=2, space="PSUM") as pm,
            tc.tile_pool(name="pc", bufs=1, space="PSUM") as pc,
            tc.tile_pool(name="expp", bufs=EXP_BUFS) as expp,
            tc.tile_pool(name="outp", bufs=OUT_BUFS) as outp,
            tc.tile_pool(name="ocp", bufs=2) as ocp,
            tc.tile_pool(name="csbp", bufs=4) as csbp,
            tc.tile_pool(name="smapp", bufs=4) as smapp,
            tc.tile_pool(name="small", bufs=10) as small,
            tc.tile_pool(name="gatep", bufs=NBL + LB) as gatep,
            tc.tile_pool(name="dram", bufs=1, space="DRAM") as dram,
        ):
            # ---------------- prologue ----------------
            # hidden first (the first matmul's dependency), then the junk
            # all-reduce that pre-pays the in-NEFF collective start cost.
            h8t = []
            for kk in range(NKK):
                t = const.tile([128, 2, RH], F8, tag=f"h8t{kk}", name=f"h8t{kk}")
                # block 0-1's columns first so the first matmul isn't gated
                # on the full hidden transfer
                nc.gpsimd.dma_start(t[:, :, :256], h8d[kk][:, :, :256])
                h8t.append(t)
            for kk in range(NKK):
                nc.gpsimd.dma_start(
                    h8t[kk][:, :, 256:], h8d[kk][:, :, 256:]
                )
            dum = small.tile([128, 1], F32, tag="dum", name="dum")
            nc.gpsimd.memset(dum[:], 0.0)
            dmi = dram.tile([128, 1], F32, tag="dmi", name="dmi")
            dmo = dram.tile([128, 1], F32, tag="dmo", name="dmo")
            nc.gpsimd.dma_start(dmi[:], dum[:])
            nc.gpsimd.collective_compute(
                "AllReduce",
                mybir.AluOpType.add,
                replica_groups=[[0, 1, 2, 3], [4, 5, 6, 7]],
                ins=[dmi.opt()],
                outs=[dmo.opt()],
            )
            # the 4 MB W shard streams in column slices, kk=0 on the ACT
            # ring and kk=1 on the SP ring, so block 0's chunks land just
            # ahead of the PE's consumption
            w8t = []
            for kk in range(NKK):
                t = const.tile([128, 2, VSP], F8, tag=f"w8t{kk}", name=f"w8t{kk}")
                w8t.append(t)
            w_slices = [(0, 512), (512, 2048), (2048, 4096), (4096, 6144),
                        (6144, VSP)]
            for (vo, ve) in w_slices:
                for kk in range(NKK):
                    eng = nc.scalar if kk == 0 else nc.sync
                    eng.dma_start(
                        w8t[kk][:, :, vo:ve], w8d[kk][:, :, vo:ve]
                    )
            # copy-branch inputs on the SP ring (needed ~20us in)
            h16_t = []
            wc_t = []
            attnT_t = []
            ebb_t = []
            for k in range(KC):
                th = const.tile([128, LB * RB], F16, tag=f"h16_{k}", name=f"h16_{k}")
                nc.sync.dma_start(th[:], h16d[k * 128:(k + 1) * 128, :])
                h16_t.append(th)
                tw = const.tile([128, 1], F16, tag=f"wc16_{k}", name=f"wc16_{k}")
                nc.sync.dma_start(tw[:], wc16d[k * 128:(k + 1) * 128, :])
                wc_t.append(tw)
            for k in range(4):
                sk = SCH[k]
                t = const.tile([128, LB * RB], F16, tag=f"attnT{k}", name=f"attnT{k}")
                nc.sync.dma_start(t[:sk, :], attnT[SOFF[k]:SOFF[k] + sk, :])
                attnT_t.append(t)
            if with_bias:
                for i in range(NVC):
                    t = const.tile([128, RW[i]], F32, tag=f"ebb{i}", name=f"ebb{i}")
                    nc.sync.dma_start(t[:], ebb[:, VOFF[i]:VOFF[i] + RW[i]])
                    ebb_t.append(t)

            # ---------------- main loop ----------------
            exp_tiles = [[None] * NVC for _ in range(NBL)]
            sg_tiles = [None] * NG    # group local sums [128, GROUP]
            cc_out = [None] * NG      # group all-reduced sums (SBUF)

            def compute_block(jb):
                cb = slice(jb * RB, (jb + 1) * RB)
                sp = small.tile([128, NVC], F32, tag="sp", name="sp")
                for i in range(NVC):
                    n = VCH[i]
                    rw = RW[i]
                    ps = pm.tile([128, 1536], F32, tag="pm", name="pm")
                    for kk in range(NKK):
                        for (o, w) in _mm_splits(n):
                            nc.tensor.matmul(
                                ps[:, o:o + w],
                                h8t[kk][:, :, cb],
                                w8t[kk][:, :, VOFF[i] + o:VOFF[i] + o + w],
                                start=(kk == 0), stop=(kk == NKK - 1),
                                perf_mode=DR,
                            )
                    ex = expp.tile([128, 1536], F16, tag="exp", name="exp")
                    if with_bias:
                        nc.scalar.activation(
                            ex[:, :rw], ps[:, :rw],
                            mybir.ActivationFunctionType.Exp, scale=INV,
                        )
                        nc.vector.tensor_tensor(
                            ex[:, :rw], ex[:, :rw], ebb_t[i][:, :rw],
                            mybir.AluOpType.mult,
                        )
                        nc.vector.reduce_sum(
                            sp[:, i:i + 1], ex[:, :rw],
                            axis=mybir.AxisListType.X,
                        )
                    elif i in ACT_ACC:
                        nc.scalar.activation(
                            ex[:, :rw], ps[:, :rw],
                            mybir.ActivationFunctionType.Exp, scale=INV,
                            accum_out=sp[:, i:i + 1],
                        )
                    else:
                        nc.scalar.activation(
                            ex[:, :rw], ps[:, :rw],
                            mybir.ActivationFunctionType.Exp, scale=INV,
                        )
                        nc.vector.reduce_sum(
                            sp[:, i:i + 1], ex[:, :rw],
                            axis=mybir.AxisListType.X,
                        )
                    if i == NVC - 1:
                        # folded copy-gate numerator ep = exp(h . w_copy)
                        nc.scalar.activation(
                            ex[:, GCOL:GCOL + 1], ps[:, GCOL:GCOL + 1],
                            mybir.ActivationFunctionType.Exp, scale=INV,
                        )
                    exp_tiles[jb][i] = ex
                g, j = GRPOF[jb]
                nc.vector.reduce_sum(
                    sg_tiles[g][:, j:j + 1], sp[:], axis=mybir.AxisListType.X
                )

            def scale_block(jb):
                g, j = GRPOF[jb]
                sgl = cc_out[g]
                ept = exp_tiles[jb][NVC - 1][:, GCOL:GCOL + 1]
                # m = (1-gate)/S = 1 / ((1 + ep*e^{b_copy}) * (S - pad_corr))
                upl = small.tile([128, 1], F32, tag="upl", name="upl")
                if ebc == 1.0:
                    nc.vector.tensor_scalar_add(upl[:], ept, 1.0)
                else:
                    nc.vector.tensor_scalar(
                        upl[:], ept, ebc, 1.0,
                        mybir.AluOpType.mult, mybir.AluOpType.add,
                    )
                corr = small.tile([128, 1], F32, tag="corr", name="corr")
                nc.vector.tensor_scalar_add(corr[:], sgl[:, j:j + 1], -pad_corr)
                v = small.tile([128, 1], F32, tag="v", name="v")
                nc.vector.tensor_scalar(
                    v[:], corr[:], upl[:], None, mybir.AluOpType.mult
                )
                m = small.tile([128, 1], F32, tag="m", name="m")
                nc.vector.reciprocal(m[:], v[:])
                # scale exp chunks into bf16 staging tiles, 2 stores per block
                for half in range(2):
                    hn = 4096 if half == 0 else VS - 4096
                    ot = outp.tile([128, 4096], BF16, tag="ot", name="ot")
                    for i in range(NVC):
                        for (so, hf, do, wd) in SEG[i]:
                            if hf != half:
                                continue
                            nc.vector.tensor_scalar(
                                ot[:, do:do + wd],
                                exp_tiles[jb][i][:, so:so + wd], m[:], None,
                                mybir.AluOpType.mult,
                            )
                    nc.sync.dma_start(
                        og[jb * RB:(jb + 1) * RB, 4096 * half:4096 * half + hn],
                        ot[:, :hn],
                    )

            # ---------------- copy branch (batch-sharded) ----------------
            def emit_copy_branch():
                for l in range(LB):
                    tb = slice(l * RB, (l + 1) * RB)
                    # one [128,608] psum tile: scatter matmul in [:600],
                    # fp16 gate dot in [600:601] (disjoint psum regions)
                    cps = pc.tile([128, 608], F32, tag="cp", name="cp")
                    for k in range(KC):
                        nc.tensor.matmul(
                            cps[:, 600:601], h16_t[k][:, tb], wc_t[k][:],
                            start=(k == 0), stop=(k == KC - 1),
                        )
                    el = gatep.tile([128, 1], F32, tag="el", name="el")
                    nc.scalar.activation(
                        el[:], cps[:, 600:601],
                        mybir.ActivationFunctionType.Exp,
                        bias=-float(b_copy), scale=-1.0,
                    )
                    ul = gatep.tile([128, 1], F32, tag="ul", name="ul")
                    nc.vector.tensor_scalar_add(ul[:], el[:], 1.0)
                    gl = gatep.tile([128, 1], F32, tag="gl", name="gl")
                    nc.vector.reciprocal(gl[:], ul[:])
                    for k in range(4):
                        sk = SCH[k]
                        sm = smapp.tile([128, CV], F16, tag="sm", name="sm")
                        nc.scalar.dma_start(
                            sm[:sk, :], smap[l, SOFF[k]:SOFF[k] + sk, :]
                        )
                        for (o, w) in _mm_splits(CV):
                            nc.tensor.matmul(
                                cps[:, o:o + w],
                                attnT_t[k][:sk, tb],
                                sm[:sk, o:o + w],
                                start=(k == 0), stop=(k == 3),
                            )
                    # drain PSUM promptly via an ACT copy so the next l's
                    # matmuls don't serialize behind gl (whose DVE ops queue
                    # after collective-blocked scale ops); the f16 roundtrip
                    # costs ~5e-4 relative on the copy branch
                    csb = csbp.tile([128, CV], F16, tag="csb", name="csb")
                    nc.scalar.activation(
                        csb[:], cps[:, :CV],
                        mybir.ActivationFunctionType.Copy,
                    )
                    oct_ = ocp.tile([128, CV], F32, tag="oct", name="oct")
                    nc.vector.tensor_scalar(
                        oct_[:], csb[:], gl[:], None, mybir.AluOpType.mult
                    )
                    nc.sync.dma_start(oc[tb, :], oct_[:])

            # Scales are deferred by ONE group: the DVE queue is strict FIFO,
            # so if scales(g) — which wait on all-reduce g — were enqueued
            # before reduces(g+1), a slow collective would head-of-line-block
            # the partial sums that feed the NEXT all-reduce and serialize
            # the whole chain.  Deferring keeps every reduce ahead of any
            # collective-blocked scale (exp pool sized for 2 groups).
            for g in range(NG):
                gn = GS[g]
                sg_tiles[g] = small.tile([128, gn], F32, tag="sg", name="sg")
                for j in range(gn):
                    compute_block(GOFF[g] + j)
                if g == 2:
                    # emitted mid-stream (not at g=0): its smap/attn inputs
                    # arrive ~20-30us in, and emitting it early would
                    # head-of-line-block the Tensor queue on those DMAs
                    emit_copy_branch()
                # deferred scales BEFORE this group's all-reduce dispatch:
                # keeps the gpsimd ring order readback -> stores -> next cin
                # (no og store ever queues behind a later AR readback, which
                # would cycle through the exp pool into a deadlock)
                if g > 0:
                    for j in range(GS[g - 1]):
                        scale_block(GOFF[g - 1] + j)
                # all-reduce this group's local sums across the 4 vocab shards
                cin = dram.tile([128, gn], F32, tag=f"cin{g}", name=f"cin{g}")
                cout = dram.tile([128, gn], F32, tag=f"cout{g}", name=f"cout{g}")
                nc.gpsimd.dma_start(cin[:], sg_tiles[g][:])
                nc.gpsimd.collective_compute(
                    "AllReduce",
                    mybir.AluOpType.add,
                    replica_groups=[[0, 1, 2, 3], [4, 5, 6, 7]],
                    ins=[cin.opt()],
                    outs=[cout.opt()],
                )
                sgl = small.tile([128, gn], F32, tag="sgl", name="sgl")
                nc.gpsimd.dma_start(sgl[:], cout[:])
                cc_out[g] = sgl
            for j in range(GS[NG - 1]):
                scale_block(GOFF[NG - 1] + j)


    nc.finalize()
    return nc


_warmed_up = False


def _warmup_collectives():
    """Run a minimal NEFF with one AllReduce so the collective channel
    (ncfw firmware / TOPSP) is warm before the main kernel executes —
    the first collective after boot costs ~60-75us of start latency."""
    global _warmed_up
    if _warmed_up:
        return
    nc = bacc.Bacc()
    x = nc.dram_tensor("x", [128, 4], F32, kind="ExternalInput")
    y = nc.dram_tensor("y", [128, 4], F32, kind="ExternalOutput")
    with tile.TileContext(nc) as tc:
        with (
            tc.tile_pool(name="sb", bufs=2) as sb,
            tc.tile_pool(name="dr", bufs=2, space="DRAM") as dr,
        ):
            t = sb.tile([128, 4], F32, tag="t", name="t")
            nc.sync.dma_start(t[:], x[:])
            bi = dr.tile([128, 4], F32, tag="bi", name="bi")
            bo = dr.tile([128, 4], F32, tag="bo", name="bo")
            nc.sync.dma_start(bi[:], t[:])
            nc.gpsimd.collective_compute(
                "AllReduce",
                mybir.AluOpType.add,
                replica_groups=[[0, 1, 2, 3], [4, 5, 6, 7]],
                ins=[bi.opt()],
                outs=[bo.opt()],
            )
            t2 = sb.tile([128, 4], F32, tag="t2", name="t2")
            nc.sync.dma_start(t2[:], bo[:])
            nc.sync.dma_start(y[:], t2[:])
    nc.finalize()
    z = np.zeros((128, 4), np.float32)
    run_bass_kernel_spmd(nc, [{"x": z}] * NCORES, core_ids=list(range(NCORES)))
    _warmed_up = True


def _pair_pack(a):
    """[256, N] -> [128, 2, N] with row k*128+p -> [p, k, :]."""
    n = a.shape[1]
    return np.ascontiguousarray(a.reshape(2, 128, n).transpose(1, 0, 2))


def kernel(hidden, copy_attn, src_map, W, b, w_copy, b_copy, _trace=False):
    hidden = np.asarray(hidden, np.float32)
    copy_attn = np.asarray(copy_attn, np.float32)
    src_map = np.asarray(src_map, np.float32)
    W = np.asarray(W, np.float32)
    b = np.asarray(b, np.float32)
    w_copy = np.asarray(w_copy, np.float32)
    b_copy_f = float(np.asarray(b_copy))

    with_bias = bool(np.any(b != 0.0))
    pad_corr = float(np.exp(b[PAD])) if with_bias else 1.0

    # host-side shard prep (layout only; W[PAD,:] is dead data in the ref)
    Wz = W.copy()
    Wz[PAD, :] = 0.0
    W8 = (np.ascontiguousarray(Wz.T) * SW).astype(ml_dtypes.float8_e4m3)  # [D, V]
    wc8 = (w_copy.reshape(D, 1) * SW).astype(ml_dtypes.float8_e4m3)
    hT_f = np.ascontiguousarray(hidden.T)                            # [D, R] f32
    h8 = (hT_f * SH).astype(ml_dtypes.float8_e4m3)
    h16 = hT_f.astype(np.float16)
    wc16 = w_copy.reshape(D, 1).astype(np.float16)
    attnT_full = np.ascontiguousarray(copy_attn.T).astype(np.float16)  # [S, R]
    smap16 = src_map.astype(np.float16)                              # [B,S,CV]

    _warmup_collectives()
    nc = build_program(with_bias, b_copy_f, pad_corr)

    in_maps = []
    zpad = np.zeros((D, 15), dtype=ml_dtypes.float8_e4m3)
    for c in range(NCORES):
        h, q = divmod(c, NQ)
        rows = slice(h * RH, (h + 1) * RH)
        crows = slice(c * LB * RB, (c + 1) * LB * RB)
        w8full = np.concatenate([W8[:, q * VS:(q + 1) * VS], wc8, zpad], axis=1)
        m = {
            "h16": np.ascontiguousarray(h16[:, crows]),
            "wc16": wc16,
            "attnT": np.ascontiguousarray(attnT_full[:, crows]),
            "smap": np.ascontiguousarray(smap16[c * LB:(c + 1) * LB]),
        }
        for kk in range(NKK):
            m[f"h8_{kk}"] = _pair_pack(h8[kk * 256:(kk + 1) * 256, rows])
            m[f"w8_{kk}"] = _pair_pack(w8full[kk * 256:(kk + 1) * 256, :])
        if with_bias:
            eb = np.exp(b[q * VS:(q + 1) * VS].astype(np.float64)).astype(
                np.float32
            )
            m["ebb"] = np.ascontiguousarray(
                np.broadcast_to(eb[None, :], (128, VS))
            )
        in_maps.append(m)

    trace_cores = None
    if os.environ.get("TRACE_ALL_CORES"):
        trace_cores = list(range(NCORES))
    res = run_bass_kernel_spmd(
        nc, in_maps, core_ids=list(range(NCORES)), trace=_trace,
        trace_cores=trace_cores,
    )

    out = np.empty((R, V + CV), np.float32)
    for c in range(NCORES):
        h, q = divmod(c, NQ)
        out[h * RH:(h + 1) * RH, q * VS:(q + 1) * VS] = (
            res.results[c]["og"].astype(np.float32)
        )
        out[c * LB * RB:(c + 1) * LB * RB, V:] = res.results[c]["oc"]
    out[:, PAD] = 0.0

    if _trace:
        kernel.last_results = res
    return out


kernel.last_results = None


# revision 28
# speedup vs baseline: 1.0892x; 1.0892x over previous
"""Bass/Trainium2 kernel for nn_CopyGenerator (8-core SPMD).

Sharding: 4-way vocab (tensor parallel) x 2-way rows (data parallel).
Core c = 4*h + q owns rows [2048h, 2048h+2048) and vocab columns
[8000q, 8000q+8000).  The softmax denominator needs a cross-vocab-shard
sum: one AllReduce over 4 ranks per tapered group of row-blocks (GS),
in two independent replica groups ([[0,1,2,3],[4,5,6,7]]) that pipeline
behind compute.  The copy branch stays batch-sharded 8 ways (4
batches/core).  A tiny warmup NEFF with one AllReduce runs first, and
the main NEFF issues a junk AllReduce immediately after dispatching
its input loads: even with the warmup, the first collective inside a
NEFF pays ~12-18us of channel start that would otherwise stall the
group-0 softmax scale (and, through exp-pool backpressure, the PE).

The big matmul runs in fp8 e4m3 with perf_mode=DoubleRow (2 fp8
weights/PE cell, K=256 per pass -> ~2x bf16 FLOP rate).  hidden is
scaled x16 and W x64 before the e4m3 cast so the bulk of both
distributions sits in the normal range (min normal 2^-6); the Exp
activation un-scales via its fp32 `scale` operand (1/1024).  fp8
quantization adds ~3-4% relative noise to individual softmax probs,
which is far inside the 2e-2 budget because gen-branch probs (~2e-4)
are tiny against the copy-branch absmax (~0.1) and the denominator
noise averages out over 32000 terms.

The copy-gate logit is FOLDED into the big matmul as one extra W
column (col 8000 of the padded shard): sigmoid(x) = ep/(1+ep) with
ep = exp(x) falling out of the same Exp pass, so the per-block scale
is m = (1-gate)/S = 1/((1+ep*e^{b_copy}) * (S_allreduce - pad_corr)).
This removes 64 N=1 gate matmuls (each paying a full LDWEIGHTS) from
the PE stream.

Per 128-row block (steady state, all three engines within ~0.5us of
each other -- PE ~9.0us, ACT ~8.5us, DVE ~7.8us):
  - PE: logits into PSUM, 6 chunks of <=1536 cols ([128,1536]f32 = 3
    PSUM banks, pool of 2), 2 DoubleRow K-passes x 512-col matmuls.
  - ACT: Exp (scale=1/1024) into SBUF fp16.  Free-dim partial sums
    split between ACT accum_out (4 chunks) and DVE reduce (2 chunks):
    either engine alone would be over budget.
  - After the group all-reduce: DVE scales exp by (1-gate)/S into bf16
    staging tiles (host upcasts; probs are ~1e-4 so bf16 rounding is
    ~1e-7 absolute).
  - PAD masking: host zeroes W[PAD,:] (dead data in the reference), the
    resulting constant exp(0)=1 is subtracted from the reduced sum, and
    the host zeroes output column PAD.
  - Copy branch: fp16 matmul (one-hot src_map is exact in fp16); its
    gate ALSO needs ~1e-3 accuracy (it multiplies values that ARE the
    output absmax) so it gets its own fp16 dot product -- fp16 is
    ~5e-4 there, e4m3 would be ~2-4%.  Its PSUM tile carries both the
    600-col scatter matmul and the 1-col gate dot (disjoint psum
    regions of one [128,608] tile) so the main loop can use 6 banks.
"""

import os
import sys

for _p in ("/opt/trn_rl_repo", "/root/.axon_site/_ro/trn_rl_repo"):
    if os.path.isdir(_p) and _p not in sys.path:
        sys.path.insert(0, _p)

import numpy as np
import ml_dtypes

import concourse.bacc as bacc
import concourse.tile as tile
from concourse import mybir
from concourse.bass_utils import run_bass_kernel_spmd

# ---------------------------------------------------------------------------
# Problem dimensions (hardcoded per spec)
# ---------------------------------------------------------------------------
B, T, S, V, CV, D = 32, 128, 400, 32000, 600, 512
PAD = 1
NCORES = 8
NQ = 4                    # vocab shards
NH = 2                    # row halves
R = B * T                 # 4096 rows
VS = V // NQ              # 8000 vocab columns per core
VSP = VS + 16             # padded shard: col VS = w_copy, cols VS+1.. = 0
RH = R // NH              # 2048 rows per core
RB = 128                  # rows per block (= one batch: T == 128)
NBL = RH // RB            # 16 row blocks per core
# tapered all-reduce groups: small first group fills the pipeline before the
# exp pool saturates; tiny last groups shrink the drain tail
GS = [2, 3, 3, 3, 3, 2]   # sums to NBL
NG = len(GS)
GOFF = [sum(GS[:i]) for i in range(NG)]
GRPOF = []                # block -> (group, index-in-group)
for _g, _n in enumerate(GS):
    for _j in range(_n):
        GRPOF.append((_g, _j))
LB = B // NCORES          # 4 local batches per core (copy branch)
KC = D // 128             # 4 contraction chunks = 2 DoubleRow passes
NKK = KC // 2
# vocab chunking within a block ([128,1536]f32 = 3 PSUM banks)
NVC = 6
VCH = [1536] * 5 + [336]  # matmul/psum width (last: 320 vocab + gate + pad)
RW = [1536] * 5 + [320]   # softmax width (excludes gate + pads)
VOFF = [1536 * i for i in range(NVC)]
GCOL = 320                # gate column within chunk 5
ACT_ACC = (0, 2, 4, 5)    # chunks whose partial sum runs on ACT accum_out
# chunk -> list of (src_off, half, dst_off, width) for the two output halves
SEG = []
for _c in range(NVC):
    _s = []
    for (_h0, _h1, _hf) in ((0, 4096, 0), (4096, VS, 1)):
        _lo = max(VOFF[_c], _h0)
        _hi = min(VOFF[_c] + RW[_c], _h1)
        if _hi > _lo:
            _s.append((_lo - VOFF[_c], _hf, _lo - 4096 * _hf, _hi - _lo))
    SEG.append(_s)
# s-dim chunks for the copy branch: 400 = 128+128+128+16
SCH = [128, 128, 128, 16]
SOFF = [0, 128, 256, 384]

F32 = mybir.dt.float32
F16 = mybir.dt.float16
BF16 = mybir.dt.bfloat16
F8 = mybir.dt.float8e4
DR = mybir.MatmulPerfMode.DoubleRow

# fp8 pre-scales (host multiplies before the e4m3 cast; Exp un-scales)
SH = 16.0                 # hidden scale
SW = 64.0                 # W / w_copy scale
INV = 1.0 / (SH * SW)     # 1/1024

EXP_BUFS = 39   # in-flight exp tiles ([128,1536] f16) ~ 6.5 blocks
OUT_BUFS = 3    # [128, 4096] bf16 output staging tiles (2 per block)


def _mm_splits(n):
    """Split a free-dim span into <=512 pieces aligned to 512 (PSUM banks)."""
    out = []
    off = 0
    while off < n:
        w = min(512, n - off)
        out.append((off, w))
        off += w
    return out


def build_program(with_bias: bool, b_copy: float, pad_corr: float):
    # Bacc (not plain Bass): its finalize() runs move_matmul_waits_to_ldweights
    # + generate_event_semaphores, which split multi-sem waits down to the
    # TRN2 limit of one wait per instruction — walrus rejects the IR otherwise.
    nc = bacc.Bacc()

    ebc = float(np.exp(b_copy))

    # k-pair-packed fp8 operands: [128, 2, free] per DoubleRow pass
    h8d = [nc.dram_tensor(f"h8_{kk}", [128, 2, RH], F8, kind="ExternalInput")
           for kk in range(NKK)]
    w8d = [nc.dram_tensor(f"w8_{kk}", [128, 2, VSP], F8, kind="ExternalInput")
           for kk in range(NKK)]
    h16d = nc.dram_tensor("h16", [D, LB * RB], F16, kind="ExternalInput")
    wc16d = nc.dram_tensor("wc16", [D, 1], F16, kind="ExternalInput")
    attnT = nc.dram_tensor("attnT", [S, LB * RB], F16, kind="ExternalInput")
    smap = nc.dram_tensor("smap", [LB, S, CV], F16, kind="ExternalInput")
    if with_bias:
        ebb = nc.dram_tensor("ebb", [128, VS], F32, kind="ExternalInput")

    og = nc.dram_tensor("og", [RH, VS], BF16, kind="ExternalOutput")
    oc = nc.dram_tensor("oc", [LB * RB, CV], F32, kind="ExternalOutput")

    with tile.TileContext(nc) as tc:
        with (
            tc.tile_pool(name="const", bufs=1) as const,
            tc.tile_pool(name="pm", bufs=2, space="PSUM") as pm,
            tc.tile_pool(name="pc", bufs=1, space="PSUM") as pc,
            tc.tile_pool(name="expp", bufs=EXP_BUFS) as expp,
            tc.tile_pool(name="outp", bufs=OUT_BUFS) as outp,
            tc.tile_pool(name="ocp", bufs=2) as ocp,
            tc.tile_pool(name="csbp", bufs=4) as csbp,
            tc.tile_pool(name="smapp", bufs=4) as smapp,
            tc.tile_pool(name="small", bufs=10) as small,
            tc.tile_pool(name="gatep", bufs=NBL + LB) as gatep,
            tc.tile_pool(name="dram", bufs=1, space="DRAM") as dram,
        ):
            # ---------------- prologue ----------------
            # hidden first (the first matmul's dependency), then the junk
            # all-reduce that pre-pays the in-NEFF collective start cost.
            h8t = []
            for kk in range(NKK):
                t = const.tile([128, 2, RH], F8, tag=f"h8t{kk}", name=f"h8t{kk}")
                # block 0-1's columns first so the first matmul isn't gated
                # on the full hidden transfer
                nc.gpsimd.dma_start(t[:, :, :256], h8d[kk][:, :, :256])
                h8t.append(t)
            for kk in range(NKK):
                nc.gpsimd.dma_start(
                    h8t[kk][:, :, 256:], h8d[kk][:, :, 256:]
                )
            # the 4 MB W shard streams in column slices, kk=0 on the ACT
            # ring and kk=1 on the SP ring, so block 0's chunks land just
            # ahead of the PE's consumption
            w8t = []
            for kk in range(NKK):
                t = const.tile([128, 2, VSP], F8, tag=f"w8t{kk}", name=f"w8t{kk}")
                w8t.append(t)
            # low columns split across the ACT/SP rings, high columns on the
            # gpsimd ring (idle until the first collective ~45us in): three
            # rings keep the w8 feed ahead of the PE through blocks 0-2
            w_slices = [(0, 512), (512, 2048), (2048, 4096)]
            for (vo, ve) in w_slices:
                for kk in range(NKK):
                    eng = nc.scalar if kk == 0 else nc.sync
                    eng.dma_start(
                        w8t[kk][:, :, vo:ve], w8d[kk][:, :, vo:ve]
                    )
            for kk in range(NKK):
                nc.gpsimd.dma_start(
                    w8t[kk][:, :, 4096:6144], w8d[kk][:, :, 4096:6144]
                )
                nc.gpsimd.dma_start(
                    w8t[kk][:, :, 6144:VSP], w8d[kk][:, :, 6144:VSP]
                )
            dum = small.tile([128, 1], F32, tag="dum", name="dum")
            nc.gpsimd.memset(dum[:], 0.0)
            dmi = dram.tile([128, 1], F32, tag="dmi", name="dmi")
            dmo = dram.tile([128, 1], F32, tag="dmo", name="dmo")
            nc.gpsimd.dma_start(dmi[:], dum[:])
            nc.gpsimd.collective_compute(
                "AllReduce",
                mybir.AluOpType.add,
                replica_groups=[[0, 1, 2, 3], [4, 5, 6, 7]],
                ins=[dmi.opt()],
                outs=[dmo.opt()],
            )
            # copy-branch inputs on the SP ring (needed ~20us in)
            h16_t = []
            wc_t = []
            attnT_t = []
            ebb_t = []
            for k in range(KC):
                th = const.tile([128, LB * RB], F16, tag=f"h16_{k}", name=f"h16_{k}")
                nc.sync.dma_start(th[:], h16d[k * 128:(k + 1) * 128, :])
                h16_t.append(th)
                tw = const.tile([128, 1], F16, tag=f"wc16_{k}", name=f"wc16_{k}")
                nc.sync.dma_start(tw[:], wc16d[k * 128:(k + 1) * 128, :])
                wc_t.append(tw)
            for k in range(4):
                sk = SCH[k]
                t = const.tile([128, LB * RB], F16, tag=f"attnT{k}", name=f"attnT{k}")
                nc.sync.dma_start(t[:sk, :], attnT[SOFF[k]:SOFF[k] + sk, :])
                attnT_t.append(t)
            if with_bias:
                for i in range(NVC):
                    t = const.tile([128, RW[i]], F32, tag=f"ebb{i}", name=f"ebb{i}")
                    nc.sync.dma_start(t[:], ebb[:, VOFF[i]:VOFF[i] + RW[i]])
                    ebb_t.append(t)

            # ---------------- main loop ----------------
            exp_tiles = [[None] * NVC for _ in range(NBL)]
            sg_tiles = [None] * NG    # group local sums [128, GROUP]
            cc_out = [None] * NG      # group all-reduced sums (SBUF)

            def compute_block(jb):
                cb = slice(jb * RB, (jb + 1) * RB)
                sp = small.tile([128, NVC], F32, tag="sp", name="sp")
                for i in range(NVC):
                    n = VCH[i]
                    rw = RW[i]
                    ps = pm.tile([128, 1536], F32, tag="pm", name="pm")
                    for kk in range(NKK):
                        for (o, w) in _mm_splits(n):
                            nc.tensor.matmul(
                                ps[:, o:o + w],
                                h8t[kk][:, :, cb],
                                w8t[kk][:, :, VOFF[i] + o:VOFF[i] + o + w],
                                start=(kk == 0), stop=(kk == NKK - 1),
                                perf_mode=DR,
                            )
                    ex = expp.tile([128, 1536], F16, tag="exp", name="exp")
                    if with_bias:
                        nc.scalar.activation(
                            ex[:, :rw], ps[:, :rw],
                            mybir.ActivationFunctionType.Exp, scale=INV,
                        )
                        nc.vector.tensor_tensor(
                            ex[:, :rw], ex[:, :rw], ebb_t[i][:, :rw],
                            mybir.AluOpType.mult,
                        )
                        nc.vector.reduce_sum(
                            sp[:, i:i + 1], ex[:, :rw],
                            axis=mybir.AxisListType.X,
                        )
                    elif i in ACT_ACC:
                        nc.scalar.activation(
                            ex[:, :rw], ps[:, :rw],
                            mybir.ActivationFunctionType.Exp, scale=INV,
                            accum_out=sp[:, i:i + 1],
                        )
                    else:
                        nc.scalar.activation(
                            ex[:, :rw], ps[:, :rw],
                            mybir.ActivationFunctionType.Exp, scale=INV,
                        )
                        nc.vector.reduce_sum(
                            sp[:, i:i + 1], ex[:, :rw],
                            axis=mybir.AxisListType.X,
                        )
                    if i == NVC - 1:
                        # folded copy-gate numerator ep = exp(h . w_copy)
                        nc.scalar.activation(
                            ex[:, GCOL:GCOL + 1], ps[:, GCOL:GCOL + 1],
                            mybir.ActivationFunctionType.Exp, scale=INV,
                        )
                    exp_tiles[jb][i] = ex
                g, j = GRPOF[jb]
                nc.vector.reduce_sum(
                    sg_tiles[g][:, j:j + 1], sp[:], axis=mybir.AxisListType.X
                )

            def scale_block(jb):
                g, j = GRPOF[jb]
                sgl = cc_out[g]
                ept = exp_tiles[jb][NVC - 1][:, GCOL:GCOL + 1]
                # m = (1-gate)/S = 1 / ((1 + ep*e^{b_copy}) * (S - pad_corr))
                upl = small.tile([128, 1], F32, tag="upl", name="upl")
                if ebc == 1.0:
                    nc.vector.tensor_scalar_add(upl[:], ept, 1.0)
                else:
                    nc.vector.tensor_scalar(
                        upl[:], ept, ebc, 1.0,
                        mybir.AluOpType.mult, mybir.AluOpType.add,
                    )
                corr = small.tile([128, 1], F32, tag="corr", name="corr")
                nc.vector.tensor_scalar_add(corr[:], sgl[:, j:j + 1], -pad_corr)
                v = small.tile([128, 1], F32, tag="v", name="v")
                nc.vector.tensor_scalar(
                    v[:], corr[:], upl[:], None, mybir.AluOpType.mult
                )
                m = small.tile([128, 1], F32, tag="m", name="m")
                nc.vector.reciprocal(m[:], v[:])
                # scale exp chunks into bf16 staging tiles, 2 stores per block
                for half in range(2):
                    hn = 4096 if half == 0 else VS - 4096
                    ot = outp.tile([128, 4096], BF16, tag="ot", name="ot")
                    for i in range(NVC):
                        for (so, hf, do, wd) in SEG[i]:
                            if hf != half:
                                continue
                            nc.vector.tensor_scalar(
                                ot[:, do:do + wd],
                                exp_tiles[jb][i][:, so:so + wd], m[:], None,
                                mybir.AluOpType.mult,
                            )
                    nc.sync.dma_start(
                        og[jb * RB:(jb + 1) * RB, 4096 * half:4096 * half + hn],
                        ot[:, :hn],
                    )

            # ---------------- copy branch (batch-sharded) ----------------
            def emit_copy_branch():
                for l in range(LB):
                    tb = slice(l * RB, (l + 1) * RB)
                    # one [128,608] psum tile: scatter matmul in [:600],
                    # fp16 gate dot in [600:601] (disjoint psum regions)
                    cps = pc.tile([128, 608], F32, tag="cp", name="cp")
                    for k in range(KC):
                        nc.tensor.matmul(
                            cps[:, 600:601], h16_t[k][:, tb], wc_t[k][:],
                            start=(k == 0), stop=(k == KC - 1),
                        )
                    el = gatep.tile([128, 1], F32, tag="el", name="el")
                    nc.scalar.activation(
                        el[:], cps[:, 600:601],
                        mybir.ActivationFunctionType.Exp,
                        bias=-float(b_copy), scale=-1.0,
                    )
                    ul = gatep.tile([128, 1], F32, tag="ul", name="ul")
                    nc.vector.tensor_scalar_add(ul[:], el[:], 1.0)
                    gl = gatep.tile([128, 1], F32, tag="gl", name="gl")
                    nc.vector.reciprocal(gl[:], ul[:])
                    for k in range(4):
                        sk = SCH[k]
                        sm = smapp.tile([128, CV], F16, tag="sm", name="sm")
                        nc.scalar.dma_start(
                            sm[:sk, :], smap[l, SOFF[k]:SOFF[k] + sk, :]
                        )
                        for (o, w) in _mm_splits(CV):
                            nc.tensor.matmul(
                                cps[:, o:o + w],
                                attnT_t[k][:sk, tb],
                                sm[:sk, o:o + w],
                                start=(k == 0), stop=(k == 3),
                            )
                    # drain PSUM promptly via an ACT copy so the next l's
                    # matmuls don't serialize behind gl (whose DVE ops queue
                    # after collective-blocked scale ops); the f16 roundtrip
                    # costs ~5e-4 relative on the copy branch
                    csb = csbp.tile([128, CV], F16, tag="csb", name="csb")
                    nc.scalar.activation(
                        csb[:], cps[:, :CV],
                        mybir.ActivationFunctionType.Copy,
                    )
                    oct_ = ocp.tile([128, CV], F32, tag="oct", name="oct")
                    nc.vector.tensor_scalar(
                        oct_[:], csb[:], gl[:], None, mybir.AluOpType.mult
                    )
                    nc.sync.dma_start(oc[tb, :], oct_[:])

            # Scales are deferred by ONE group: the DVE queue is strict FIFO,
            # so if scales(g) — which wait on all-reduce g — were enqueued
            # before reduces(g+1), a slow collective would head-of-line-block
            # the partial sums that feed the NEXT all-reduce and serialize
            # the whole chain.  Deferring keeps every reduce ahead of any
            # collective-blocked scale (exp pool sized for 2 groups).
            for g in range(NG):
                gn = GS[g]
                sg_tiles[g] = small.tile([128, gn], F32, tag="sg", name="sg")
                for j in range(gn):
                    compute_block(GOFF[g] + j)
                if g == 2:
                    # emitted mid-stream (not at g=0): its smap/attn inputs
                    # arrive ~20-30us in, and emitting it early would
                    # head-of-line-block the Tensor queue on those DMAs
                    emit_copy_branch()
                # deferred scales BEFORE this group's all-reduce dispatch:
                # keeps the gpsimd ring order readback -> stores -> next cin
                # (no og store ever queues behind a later AR readback, which
                # would cycle through the exp pool into a deadlock)
                if g > 0:
                    for j in range(GS[g - 1]):
                        scale_block(GOFF[g - 1] + j)
                # all-reduce this group's local sums across the 4 vocab shards
                cin = dram.tile([128, gn], F32, tag=f"cin{g}", name=f"cin{g}")
                cout = dram.tile([128, gn], F32, tag=f"cout{g}", name=f"cout{g}")
                nc.gpsimd.dma_start(cin[:], sg_tiles[g][:])
                nc.gpsimd.collective_compute(
                    "AllReduce",
                    mybir.AluOpType.add,
                    replica_groups=[[0, 1, 2, 3], [4, 5, 6, 7]],
                    ins=[cin.opt()],
                    outs=[cout.opt()],
                )
                sgl = small.tile([128, gn], F32, tag="sgl", name="sgl")
                nc.gpsimd.dma_start(sgl[:], cout[:])
                cc_out[g] = sgl
            for j in range(GS[NG - 1]):
                scale_block(GOFF[NG - 1] + j)


    nc.finalize()
    return nc


_warmed_up = False


def _warmup_collectives():
    """Run a minimal NEFF with one AllReduce so the collective channel
    (ncfw firmware / TOPSP) is warm before the main kernel executes —
    the first collective after boot costs ~60-75us of start latency."""
    global _warmed_up
    if _warmed_up:
        return
    nc = bacc.Bacc()
    x = nc.dram_tensor("x", [128, 4], F32, kind="ExternalInput")
    y = nc.dram_tensor("y", [128, 4], F32, kind="ExternalOutput")
    with tile.TileContext(nc) as tc:
        with (
            tc.tile_pool(name="sb", bufs=2) as sb,
            tc.tile_pool(name="dr", bufs=2, space="DRAM") as dr,
        ):
            t = sb.tile([128, 4], F32, tag="t", name="t")
            nc.sync.dma_start(t[:], x[:])
            bi = dr.tile([128, 4], F32, tag="bi", name="bi")
            bo = dr.tile([128, 4], F32, tag="bo", name="bo")
            nc.sync.dma_start(bi[:], t[:])
            nc.gpsimd.collective_compute(
                "AllReduce",
                mybir.AluOpType.add,
                replica_groups=[[0, 1, 2, 3], [4, 5, 6, 7]],
                ins=[bi.opt()],
                outs=[bo.opt()],
            )
            t2 = sb.tile([128, 4], F32, tag="t2", name="t2")
            nc.sync.dma_start(t2[:], bo[:])
            nc.sync.dma_start(y[:], t2[:])
    nc.finalize()
    z = np.zeros((128, 4), np.float32)
    run_bass_kernel_spmd(nc, [{"x": z}] * NCORES, core_ids=list(range(NCORES)))
    _warmed_up = True


def _pair_pack(a):
    """[256, N] -> [128, 2, N] with row k*128+p -> [p, k, :]."""
    n = a.shape[1]
    return np.ascontiguousarray(a.reshape(2, 128, n).transpose(1, 0, 2))


def kernel(hidden, copy_attn, src_map, W, b, w_copy, b_copy, _trace=False):
    hidden = np.asarray(hidden, np.float32)
    copy_attn = np.asarray(copy_attn, np.float32)
    src_map = np.asarray(src_map, np.float32)
    W = np.asarray(W, np.float32)
    b = np.asarray(b, np.float32)
    w_copy = np.asarray(w_copy, np.float32)
    b_copy_f = float(np.asarray(b_copy))

    with_bias = bool(np.any(b != 0.0))
    pad_corr = float(np.exp(b[PAD])) if with_bias else 1.0

    # host-side shard prep (layout only; W[PAD,:] is dead data in the ref)
    Wz = W.copy()
    Wz[PAD, :] = 0.0
    W8 = (np.ascontiguousarray(Wz.T) * SW).astype(ml_dtypes.float8_e4m3)  # [D, V]
    wc8 = (w_copy.reshape(D, 1) * SW).astype(ml_dtypes.float8_e4m3)
    hT_f = np.ascontiguousarray(hidden.T)                            # [D, R] f32
    h8 = (hT_f * SH).astype(ml_dtypes.float8_e4m3)
    h16 = hT_f.astype(np.float16)
    wc16 = w_copy.reshape(D, 1).astype(np.float16)
    attnT_full = np.ascontiguousarray(copy_attn.T).astype(np.float16)  # [S, R]
    smap16 = src_map.astype(np.float16)                              # [B,S,CV]

    _warmup_collectives()
    nc = build_program(with_bias, b_copy_f, pad_corr)

    in_maps = []
    zpad = np.zeros((D, 15), dtype=ml_dtypes.float8_e4m3)
    for c in range(NCORES):
        h, q = divmod(c, NQ)
        rows = slice(h * RH, (h + 1) * RH)
        crows = slice(c * LB * RB, (c + 1) * LB * RB)
        w8full = np.concatenate([W8[:, q * VS:(q + 1) * VS], wc8, zpad], axis=1)
        m = {
            "h16": np.ascontiguousarray(h16[:, crows]),
            "wc16": wc16,
            "attnT": np.ascontiguousarray(attnT_full[:, crows]),
            "smap": np.ascontiguousarray(smap16[c * LB:(c + 1) * LB]),
        }
        for kk in range(NKK):
            m[f"h8_{kk}"] = _pair_pack(h8[kk * 256:(kk + 1) * 256, rows])
            m[f"w8_{kk}"] = _pair_pack(w8full[kk * 256:(kk + 1) * 256, :])
        if with_bias:
            eb = np.exp(b[q * VS:(q + 1) * VS].astype(np.float64)).astype(
                np.float32
            )
            m["ebb"] = np.ascontiguousarray(
                np.broadcast_to(eb[None, :], (128, VS))
            )
        in_maps.append(m)

    trace_cores = None
    if os.environ.get("TRACE_ALL_CORES"):
        trace_cores = list(range(NCORES))
    res = run_bass_kernel_spmd(
        nc, in_maps, core_ids=list(range(NCORES)), trace=_trace,
        trace_cores=trace_cores,
    )

    out = np.empty((R, V + CV), np.float32)
    for c in range(NCORES):
        h, q = divmod(c, NQ)
        out[h * RH:(h + 1) * RH, q * VS:(q + 1) * VS] = (
            res.results[c]["og"].astype(np.float32)
        )
        out[c * LB * RB:(c + 1) * LB * RB, V:] = res.results[c]["oc"]
    out[:, PAD] = 0.0

    if _trace:
        kernel.last_results = res
    return out


kernel.last_results = None


# revision 29
# speedup vs baseline: 1.1059x; 1.0153x over previous
"""Bass/Trainium2 kernel for nn_CopyGenerator (8-core SPMD).

Sharding: 4-way vocab (tensor parallel) x 2-way rows (data parallel).
Core c = 4*h + q owns rows [2048h, 2048h+2048) and vocab columns
[8000q, 8000q+8000).  The softmax denominator needs a cross-vocab-shard
sum: one AllReduce over 4 ranks per tapered group of row-blocks (GS),
in two independent replica groups ([[0,1,2,3],[4,5,6,7]]) that pipeline
behind compute.  The copy branch stays batch-sharded 8 ways (4
batches/core).  A tiny warmup NEFF with one AllReduce runs first, and
the main NEFF issues a junk AllReduce immediately after dispatching
its input loads: even with the warmup, the first collective inside a
NEFF pays ~12-18us of channel start that would otherwise stall the
group-0 softmax scale (and, through exp-pool backpressure, the PE).

The big matmul runs in fp8 e4m3 with perf_mode=DoubleRow (2 fp8
weights/PE cell, K=256 per pass -> ~2x bf16 FLOP rate).  hidden is
scaled x16 and W x64 before the e4m3 cast so the bulk of both
distributions sits in the normal range (min normal 2^-6); the Exp
activation un-scales via its fp32 `scale` operand (1/1024).  fp8
quantization adds ~3-4% relative noise to individual softmax probs,
which is far inside the 2e-2 budget because gen-branch probs (~2e-4)
are tiny against the copy-branch absmax (~0.1) and the denominator
noise averages out over 32000 terms.

The copy-gate logit is FOLDED into the big matmul as one extra W
column (col 8000 of the padded shard): sigmoid(x) = ep/(1+ep) with
ep = exp(x) falling out of the same Exp pass, so the per-block scale
is m = (1-gate)/S = 1/((1+ep*e^{b_copy}) * (S_allreduce - pad_corr)).
This removes 64 N=1 gate matmuls (each paying a full LDWEIGHTS) from
the PE stream.

Per 128-row block (steady state, all three engines within ~0.5us of
each other -- PE ~9.0us, ACT ~8.5us, DVE ~7.8us):
  - PE: logits into PSUM, 6 chunks of <=1536 cols ([128,1536]f32 = 3
    PSUM banks, pool of 2), 2 DoubleRow K-passes x 512-col matmuls.
  - ACT: Exp (scale=1/1024) into SBUF fp16.  Free-dim partial sums
    split between ACT accum_out (4 chunks) and DVE reduce (2 chunks):
    either engine alone would be over budget.
  - After the group all-reduce: DVE scales exp by (1-gate)/S into bf16
    staging tiles (host upcasts; probs are ~1e-4 so bf16 rounding is
    ~1e-7 absolute).
  - PAD masking: host zeroes W[PAD,:] (dead data in the reference), the
    resulting constant exp(0)=1 is subtracted from the reduced sum, and
    the host zeroes output column PAD.
  - Copy branch: fp16 matmul (one-hot src_map is exact in fp16); its
    gate ALSO needs ~1e-3 accuracy (it multiplies values that ARE the
    output absmax) so it gets its own fp16 dot product -- fp16 is
    ~5e-4 there, e4m3 would be ~2-4%.  Its PSUM tile carries both the
    600-col scatter matmul and the 1-col gate dot (disjoint psum
    regions of one [128,608] tile) so the main loop can use 6 banks.
"""

import os
import sys

for _p in ("/opt/trn_rl_repo", "/root/.axon_site/_ro/trn_rl_repo"):
    if os.path.isdir(_p) and _p not in sys.path:
        sys.path.insert(0, _p)

import numpy as np
import ml_dtypes

import concourse.bacc as bacc
import concourse.tile as tile
from concourse import mybir
from concourse.bass_utils import run_bass_kernel_spmd

# ---------------------------------------------------------------------------
# Problem dimensions (hardcoded per spec)
# ---------------------------------------------------------------------------
B, T, S, V, CV, D = 32, 128, 400, 32000, 600, 512
PAD = 1
NCORES = 8
NQ = 4                    # vocab shards
NH = 2                    # row halves
R = B * T                 # 4096 rows
VS = V // NQ              # 8000 vocab columns per core
VSP = VS + 16             # padded shard: col VS = w_copy, cols VS+1.. = 0
RH = R // NH              # 2048 rows per core
RB = 128                  # rows per block (= one batch: T == 128)
NBL = RH // RB            # 16 row blocks per core
# tapered all-reduce groups: small first group fills the pipeline before the
# exp pool saturates; tiny last groups shrink the drain tail
GS = [2, 3, 3, 3, 3, 2]   # sums to NBL
NG = len(GS)
GOFF = [sum(GS[:i]) for i in range(NG)]
GRPOF = []                # block -> (group, index-in-group)
for _g, _n in enumerate(GS):
    for _j in range(_n):
        GRPOF.append((_g, _j))
LB = B // NCORES          # 4 local batches per core (copy branch)
KC = D // 128             # 4 contraction chunks = 2 DoubleRow passes
NKK = KC // 2
# vocab chunking within a block ([128,1536]f32 = 3 PSUM banks)
NVC = 6
VCH = [1536] * 5 + [336]  # matmul/psum width (last: 320 vocab + gate + pad)
RW = [1536] * 5 + [320]   # softmax width (excludes gate + pads)
VOFF = [1536 * i for i in range(NVC)]
GCOL = 320                # gate column within chunk 5
ACT_ACC = (0, 2, 4, 5)    # chunks whose partial sum runs on ACT accum_out
# chunk -> list of (src_off, half, dst_off, width) for the two output halves
SEG = []
for _c in range(NVC):
    _s = []
    for (_h0, _h1, _hf) in ((0, 4096, 0), (4096, VS, 1)):
        _lo = max(VOFF[_c], _h0)
        _hi = min(VOFF[_c] + RW[_c], _h1)
        if _hi > _lo:
            _s.append((_lo - VOFF[_c], _hf, _lo - 4096 * _hf, _hi - _lo))
    SEG.append(_s)
# s-dim chunks for the copy branch: 400 = 128+128+128+16
SCH = [128, 128, 128, 16]
SOFF = [0, 128, 256, 384]

F32 = mybir.dt.float32
F16 = mybir.dt.float16
BF16 = mybir.dt.bfloat16
F8 = mybir.dt.float8e4
DR = mybir.MatmulPerfMode.DoubleRow

# fp8 pre-scales (host multiplies before the e4m3 cast; Exp un-scales)
SH = 16.0                 # hidden scale
SW = 64.0                 # W / w_copy scale
INV = 1.0 / (SH * SW)     # 1/1024

EXP_BUFS = 39   # in-flight exp tiles ([128,1536] f16) ~ 6.5 blocks
OUT_BUFS = 3    # [128, 4096] bf16 output staging tiles (2 per block)


def _mm_splits(n):
    """Split a free-dim span into <=512 pieces aligned to 512 (PSUM banks)."""
    out = []
    off = 0
    while off < n:
        w = min(512, n - off)
        out.append((off, w))
        off += w
    return out


def build_program(with_bias: bool, b_copy: float, pad_corr: float):
    # Bacc (not plain Bass): its finalize() runs move_matmul_waits_to_ldweights
    # + generate_event_semaphores, which split multi-sem waits down to the
    # TRN2 limit of one wait per instruction — walrus rejects the IR otherwise.
    nc = bacc.Bacc()

    ebc = float(np.exp(b_copy))

    # k-pair-packed fp8 operands: [128, 2, free] per DoubleRow pass
    h8d = [nc.dram_tensor(f"h8_{kk}", [128, 2, RH], F8, kind="ExternalInput")
           for kk in range(NKK)]
    w8d = [nc.dram_tensor(f"w8_{kk}", [128, 2, VSP], F8, kind="ExternalInput")
           for kk in range(NKK)]
    h16d = nc.dram_tensor("h16", [D, LB * RB], F16, kind="ExternalInput")
    wc16d = nc.dram_tensor("wc16", [D, 1], F16, kind="ExternalInput")
    attnT = nc.dram_tensor("attnT", [S, LB * RB], F16, kind="ExternalInput")
    smap = nc.dram_tensor("smap", [LB, S, CV], F16, kind="ExternalInput")
    if with_bias:
        ebb = nc.dram_tensor("ebb", [128, VS], F32, kind="ExternalInput")

    og = nc.dram_tensor("og", [RH, VS], BF16, kind="ExternalOutput")
    oc = nc.dram_tensor("oc", [LB * RB, CV], F32, kind="ExternalOutput")

    with tile.TileContext(nc) as tc:
        with (
            tc.tile_pool(name="const", bufs=1) as const,
            tc.tile_pool(name="pm", bufs=2, space="PSUM") as pm,
            tc.tile_pool(name="pc", bufs=1, space="PSUM") as pc,
            tc.tile_pool(name="expp", bufs=EXP_BUFS) as expp,
            tc.tile_pool(name="outp", bufs=OUT_BUFS) as outp,
            tc.tile_pool(name="ocp", bufs=2) as ocp,
            tc.tile_pool(name="csbp", bufs=4) as csbp,
            tc.tile_pool(name="smapp", bufs=4) as smapp,
            tc.tile_pool(name="small", bufs=10) as small,
            tc.tile_pool(name="gatep", bufs=NBL + LB) as gatep,
            tc.tile_pool(name="dram", bufs=1, space="DRAM") as dram,
        ):
            # ---------------- prologue ----------------
            # hidden first (the first matmul's dependency), then the junk
            # all-reduce that pre-pays the in-NEFF collective start cost.
            h8t = []
            for kk in range(NKK):
                t = const.tile([128, 2, RH], F8, tag=f"h8t{kk}", name=f"h8t{kk}")
                # block 0-1's columns first so the first matmul isn't gated
                # on the full hidden transfer
                nc.gpsimd.dma_start(t[:, :, :256], h8d[kk][:, :, :256])
                h8t.append(t)
            for kk in range(NKK):
                nc.gpsimd.dma_start(
                    h8t[kk][:, :, 256:], h8d[kk][:, :, 256:]
                )
            # the 4 MB W shard streams in column slices, kk=0 on the ACT
            # ring and kk=1 on the SP ring, so block 0's chunks land just
            # ahead of the PE's consumption
            w8t = []
            for kk in range(NKK):
                t = const.tile([128, 2, VSP], F8, tag=f"w8t{kk}", name=f"w8t{kk}")
                w8t.append(t)
            # low columns split across the ACT/SP rings, high columns on the
            # gpsimd ring (idle until the first collective ~45us in): three
            # rings keep the w8 feed ahead of the PE through blocks 0-2
            w_slices = [(0, 512), (512, 2048), (2048, 4096), (4096, 6144)]
            for (vo, ve) in w_slices:
                for kk in range(NKK):
                    eng = nc.scalar if kk == 0 else nc.sync
                    eng.dma_start(
                        w8t[kk][:, :, vo:ve], w8d[kk][:, :, vo:ve]
                    )
            for kk in range(NKK):
                nc.gpsimd.dma_start(
                    w8t[kk][:, :, 6144:VSP], w8d[kk][:, :, 6144:VSP]
                )
            dum = small.tile([128, 1], F32, tag="dum", name="dum")
            nc.gpsimd.memset(dum[:], 0.0)
            dmi = dram.tile([128, 1], F32, tag="dmi", name="dmi")
            dmo = dram.tile([128, 1], F32, tag="dmo", name="dmo")
            nc.gpsimd.dma_start(dmi[:], dum[:])
            nc.gpsimd.collective_compute(
                "AllReduce",
                mybir.AluOpType.add,
                replica_groups=[[0, 1, 2, 3], [4, 5, 6, 7]],
                ins=[dmi.opt()],
                outs=[dmo.opt()],
            )
            # copy-branch inputs on the SP ring (needed ~20us in)
            h16_t = []
            wc_t = []
            attnT_t = []
            ebb_t = []
            for k in range(KC):
                th = const.tile([128, LB * RB], F16, tag=f"h16_{k}", name=f"h16_{k}")
                nc.sync.dma_start(th[:], h16d[k * 128:(k + 1) * 128, :])
                h16_t.append(th)
                tw = const.tile([128, 1], F16, tag=f"wc16_{k}", name=f"wc16_{k}")
                nc.sync.dma_start(tw[:], wc16d[k * 128:(k + 1) * 128, :])
                wc_t.append(tw)
            for k in range(4):
                sk = SCH[k]
                t = const.tile([128, LB * RB], F16, tag=f"attnT{k}", name=f"attnT{k}")
                nc.sync.dma_start(t[:sk, :], attnT[SOFF[k]:SOFF[k] + sk, :])
                attnT_t.append(t)
            if with_bias:
                for i in range(NVC):
                    t = const.tile([128, RW[i]], F32, tag=f"ebb{i}", name=f"ebb{i}")
                    nc.sync.dma_start(t[:], ebb[:, VOFF[i]:VOFF[i] + RW[i]])
                    ebb_t.append(t)

            # ---------------- main loop ----------------
            exp_tiles = [[None] * NVC for _ in range(NBL)]
            sg_tiles = [None] * NG    # group local sums [128, GROUP]
            cc_out = [None] * NG      # group all-reduced sums (SBUF)

            def compute_block(jb):
                cb = slice(jb * RB, (jb + 1) * RB)
                sp = small.tile([128, NVC], F32, tag="sp", name="sp")
                for i in range(NVC):
                    n = VCH[i]
                    rw = RW[i]
                    ps = pm.tile([128, 1536], F32, tag="pm", name="pm")
                    for kk in range(NKK):
                        for (o, w) in _mm_splits(n):
                            nc.tensor.matmul(
                                ps[:, o:o + w],
                                h8t[kk][:, :, cb],
                                w8t[kk][:, :, VOFF[i] + o:VOFF[i] + o + w],
                                start=(kk == 0), stop=(kk == NKK - 1),
                                perf_mode=DR,
                            )
                    ex = expp.tile([128, 1536], F16, tag="exp", name="exp")
                    if with_bias:
                        nc.scalar.activation(
                            ex[:, :rw], ps[:, :rw],
                            mybir.ActivationFunctionType.Exp, scale=INV,
                        )
                        nc.vector.tensor_tensor(
                            ex[:, :rw], ex[:, :rw], ebb_t[i][:, :rw],
                            mybir.AluOpType.mult,
                        )
                        nc.vector.reduce_sum(
                            sp[:, i:i + 1], ex[:, :rw],
                            axis=mybir.AxisListType.X,
                        )
                    elif i in ACT_ACC:
                        nc.scalar.activation(
                            ex[:, :rw], ps[:, :rw],
                            mybir.ActivationFunctionType.Exp, scale=INV,
                            accum_out=sp[:, i:i + 1],
                        )
                    else:
                        nc.scalar.activation(
                            ex[:, :rw], ps[:, :rw],
                            mybir.ActivationFunctionType.Exp, scale=INV,
                        )
                        nc.vector.reduce_sum(
                            sp[:, i:i + 1], ex[:, :rw],
                            axis=mybir.AxisListType.X,
                        )
                    if i == NVC - 1:
                        # folded copy-gate numerator ep = exp(h . w_copy)
                        nc.scalar.activation(
                            ex[:, GCOL:GCOL + 1], ps[:, GCOL:GCOL + 1],
                            mybir.ActivationFunctionType.Exp, scale=INV,
                        )
                    exp_tiles[jb][i] = ex
                g, j = GRPOF[jb]
                nc.vector.reduce_sum(
                    sg_tiles[g][:, j:j + 1], sp[:], axis=mybir.AxisListType.X
                )

            def scale_block(jb):
                g, j = GRPOF[jb]
                sgl = cc_out[g]
                ept = exp_tiles[jb][NVC - 1][:, GCOL:GCOL + 1]
                # m = (1-gate)/S = 1 / ((1 + ep*e^{b_copy}) * (S - pad_corr))
                upl = small.tile([128, 1], F32, tag="upl", name="upl")
                if ebc == 1.0:
                    nc.vector.tensor_scalar_add(upl[:], ept, 1.0)
                else:
                    nc.vector.tensor_scalar(
                        upl[:], ept, ebc, 1.0,
                        mybir.AluOpType.mult, mybir.AluOpType.add,
                    )
                corr = small.tile([128, 1], F32, tag="corr", name="corr")
                nc.vector.tensor_scalar_add(corr[:], sgl[:, j:j + 1], -pad_corr)
                v = small.tile([128, 1], F32, tag="v", name="v")
                nc.vector.tensor_scalar(
                    v[:], corr[:], upl[:], None, mybir.AluOpType.mult
                )
                m = small.tile([128, 1], F32, tag="m", name="m")
                nc.vector.reciprocal(m[:], v[:])
                # scale exp chunks into bf16 staging tiles, 2 stores per block
                for half in range(2):
                    hn = 4096 if half == 0 else VS - 4096
                    ot = outp.tile([128, 4096], BF16, tag="ot", name="ot")
                    for i in range(NVC):
                        for (so, hf, do, wd) in SEG[i]:
                            if hf != half:
                                continue
                            nc.vector.tensor_scalar(
                                ot[:, do:do + wd],
                                exp_tiles[jb][i][:, so:so + wd], m[:], None,
                                mybir.AluOpType.mult,
                            )
                    nc.sync.dma_start(
                        og[jb * RB:(jb + 1) * RB, 4096 * half:4096 * half + hn],
                        ot[:, :hn],
                    )

            # ---------------- copy branch (batch-sharded) ----------------
            def emit_copy_branch():
                for l in range(LB):
                    tb = slice(l * RB, (l + 1) * RB)
                    # one [128,608] psum tile: scatter matmul in [:600],
                    # fp16 gate dot in [600:601] (disjoint psum regions)
                    cps = pc.tile([128, 608], F32, tag="cp", name="cp")
                    for k in range(KC):
                        nc.tensor.matmul(
                            cps[:, 600:601], h16_t[k][:, tb], wc_t[k][:],
                            start=(k == 0), stop=(k == KC - 1),
                        )
                    el = gatep.tile([128, 1], F32, tag="el", name="el")
                    nc.scalar.activation(
                        el[:], cps[:, 600:601],
                        mybir.ActivationFunctionType.Exp,
                        bias=-float(b_copy), scale=-1.0,
                    )
                    ul = gatep.tile([128, 1], F32, tag="ul", name="ul")
                    nc.vector.tensor_scalar_add(ul[:], el[:], 1.0)
                    gl = gatep.tile([128, 1], F32, tag="gl", name="gl")
                    nc.vector.reciprocal(gl[:], ul[:])
                    for k in range(4):
                        sk = SCH[k]
                        sm = smapp.tile([128, CV], F16, tag="sm", name="sm")
                        nc.scalar.dma_start(
                            sm[:sk, :], smap[l, SOFF[k]:SOFF[k] + sk, :]
                        )
                        for (o, w) in _mm_splits(CV):
                            nc.tensor.matmul(
                                cps[:, o:o + w],
                                attnT_t[k][:sk, tb],
                                sm[:sk, o:o + w],
                                start=(k == 0), stop=(k == 3),
                            )
                    # drain PSUM promptly via an ACT copy so the next l's
                    # matmuls don't serialize behind gl (whose DVE ops queue
                    # after collective-blocked scale ops); the f16 roundtrip
                    # costs ~5e-4 relative on the copy branch
                    csb = csbp.tile([128, CV], F16, tag="csb", name="csb")
                    nc.scalar.activation(
                        csb[:], cps[:, :CV],
                        mybir.ActivationFunctionType.Copy,
                    )
                    oct_ = ocp.tile([128, CV], F32, tag="oct", name="oct")
                    nc.vector.tensor_scalar(
                        oct_[:], csb[:], gl[:], None, mybir.AluOpType.mult
                    )
                    nc.sync.dma_start(oc[tb, :], oct_[:])

            # Scales are deferred by ONE group: the DVE queue is strict FIFO,
            # so if scales(g) — which wait on all-reduce g — were enqueued
            # before reduces(g+1), a slow collective would head-of-line-block
            # the partial sums that feed the NEXT all-reduce and serialize
            # the whole chain.  Deferring keeps every reduce ahead of any
            # collective-blocked scale (exp pool sized for 2 groups).
            for g in range(NG):
                gn = GS[g]
                sg_tiles[g] = small.tile([128, gn], F32, tag="sg", name="sg")
                for j in range(gn):
                    compute_block(GOFF[g] + j)
                if g == 2:
                    # emitted mid-stream (not at g=0): its smap/attn inputs
                    # arrive ~20-30us in, and emitting it early would
                    # head-of-line-block the Tensor queue on those DMAs
                    emit_copy_branch()
                # deferred scales BEFORE this group's all-reduce dispatch:
                # keeps the gpsimd ring order readback -> stores -> next cin
                # (no og store ever queues behind a later AR readback, which
                # would cycle through the exp pool into a deadlock)
                if g > 0:
                    for j in range(GS[g - 1]):
                        scale_block(GOFF[g - 1] + j)
                # all-reduce this group's local sums across the 4 vocab shards
                cin = dram.tile([128, gn], F32, tag=f"cin{g}", name=f"cin{g}")
                cout = dram.tile([128, gn], F32, tag=f"cout{g}", name=f"cout{g}")
                nc.gpsimd.dma_start(cin[:], sg_tiles[g][:])
                nc.gpsimd.collective_compute(
                    "AllReduce",
                    mybir.AluOpType.add,
                    replica_groups=[[0, 1, 2, 3], [4, 5, 6, 7]],
                    ins=[cin.opt()],
                    outs=[cout.opt()],
                )
                sgl = small.tile([128, gn], F32, tag="sgl", name="sgl")
                nc.gpsimd.dma_start(sgl[:], cout[:])
                cc_out[g] = sgl
            for j in range(GS[NG - 1]):
                scale_block(GOFF[NG - 1] + j)


    nc.finalize()
    return nc


_warmed_up = False


def _warmup_collectives():
    """Run a minimal NEFF with one AllReduce so the collective channel
    (ncfw firmware / TOPSP) is warm before the main kernel executes —
    the first collective after boot costs ~60-75us of start latency."""
    global _warmed_up
    if _warmed_up:
        return
    nc = bacc.Bacc()
    x = nc.dram_tensor("x", [128, 4], F32, kind="ExternalInput")
    y = nc.dram_tensor("y", [128, 4], F32, kind="ExternalOutput")
    with tile.TileContext(nc) as tc:
        with (
            tc.tile_pool(name="sb", bufs=2) as sb,
            tc.tile_pool(name="dr", bufs=2, space="DRAM") as dr,
        ):
            t = sb.tile([128, 4], F32, tag="t", name="t")
            nc.sync.dma_start(t[:], x[:])
            bi = dr.tile([128, 4], F32, tag="bi", name="bi")
            bo = dr.tile([128, 4], F32, tag="bo", name="bo")
            nc.sync.dma_start(bi[:], t[:])
            nc.gpsimd.collective_compute(
                "AllReduce",
                mybir.AluOpType.add,
                replica_groups=[[0, 1, 2, 3], [4, 5, 6, 7]],
                ins=[bi.opt()],
                outs=[bo.opt()],
            )
            t2 = sb.tile([128, 4], F32, tag="t2", name="t2")
            nc.sync.dma_start(t2[:], bo[:])
            nc.sync.dma_start(y[:], t2[:])
    nc.finalize()
    z = np.zeros((128, 4), np.float32)
    run_bass_kernel_spmd(nc, [{"x": z}] * NCORES, core_ids=list(range(NCORES)))
    _warmed_up = True


def _pair_pack(a):
    """[256, N] -> [128, 2, N] with row k*128+p -> [p, k, :]."""
    n = a.shape[1]
    return np.ascontiguousarray(a.reshape(2, 128, n).transpose(1, 0, 2))


def kernel(hidden, copy_attn, src_map, W, b, w_copy, b_copy, _trace=False):
    hidden = np.asarray(hidden, np.float32)
    copy_attn = np.asarray(copy_attn, np.float32)
    src_map = np.asarray(src_map, np.float32)
    W = np.asarray(W, np.float32)
    b = np.asarray(b, np.float32)
    w_copy = np.asarray(w_copy, np.float32)
    b_copy_f = float(np.asarray(b_copy))

    with_bias = bool(np.any(b != 0.0))
    pad_corr = float(np.exp(b[PAD])) if with_bias else 1.0

    # host-side shard prep (layout only; W[PAD,:] is dead data in the ref)
    Wz = W.copy()
    Wz[PAD, :] = 0.0
    W8 = (np.ascontiguousarray(Wz.T) * SW).astype(ml_dtypes.float8_e4m3)  # [D, V]
    wc8 = (w_copy.reshape(D, 1) * SW).astype(ml_dtypes.float8_e4m3)
    hT_f = np.ascontiguousarray(hidden.T)                            # [D, R] f32
    h8 = (hT_f * SH).astype(ml_dtypes.float8_e4m3)
    h16 = hT_f.astype(np.float16)
    wc16 = w_copy.reshape(D, 1).astype(np.float16)
    attnT_full = np.ascontiguousarray(copy_attn.T).astype(np.float16)  # [S, R]
    smap16 = src_map.astype(np.float16)                              # [B,S,CV]

    _warmup_collectives()
    nc = build_program(with_bias, b_copy_f, pad_corr)

    in_maps = []
    zpad = np.zeros((D, 15), dtype=ml_dtypes.float8_e4m3)
    for c in range(NCORES):
        h, q = divmod(c, NQ)
        rows = slice(h * RH, (h + 1) * RH)
        crows = slice(c * LB * RB, (c + 1) * LB * RB)
        w8full = np.concatenate([W8[:, q * VS:(q + 1) * VS], wc8, zpad], axis=1)
        m = {
            "h16": np.ascontiguousarray(h16[:, crows]),
            "wc16": wc16,
            "attnT": np.ascontiguousarray(attnT_full[:, crows]),
            "smap": np.ascontiguousarray(smap16[c * LB:(c + 1) * LB]),
        }
        for kk in range(NKK):
            m[f"h8_{kk}"] = _pair_pack(h8[kk * 256:(kk + 1) * 256, rows])
            m[f"w8_{kk}"] = _pair_pack(w8full[kk * 256:(kk + 1) * 256, :])
        if with_bias:
            eb = np.exp(b[q * VS:(q + 1) * VS].astype(np.float64)).astype(
                np.float32
            )
            m["ebb"] = np.ascontiguousarray(
                np.broadcast_to(eb[None, :], (128, VS))
            )
        in_maps.append(m)

    trace_cores = None
    if os.environ.get("TRACE_ALL_CORES"):
        trace_cores = list(range(NCORES))
    res = run_bass_kernel_spmd(
        nc, in_maps, core_ids=list(range(NCORES)), trace=_trace,
        trace_cores=trace_cores,
    )

    out = np.empty((R, V + CV), np.float32)
    for c in range(NCORES):
        h, q = divmod(c, NQ)
        out[h * RH:(h + 1) * RH, q * VS:(q + 1) * VS] = (
            res.results[c]["og"].astype(np.float32)
        )
        out[c * LB * RB:(c + 1) * LB * RB, V:] = res.results[c]["oc"]
    out[:, PAD] = 0.0

    if _trace:
        kernel.last_results = res
    return out


kernel.last_results = None
